# revision 11
# baseline (speedup 1.0000x reference)
"""DeepAR (2-layer LSTM, B=1024, W=288, H=128) forward on 8 Trainium2 cores.

Pure data-parallel: batch 1024 -> 128 per core; weights replicated.

Device layout: (feature = partitions, batch = free).  sigmoid(x) =
(tanh(x/2)+1)/2 so ONE tanh table covers all gates; i/f/o weight rows are
pre-halved.  States: C = 2c, H = 2h (h-consuming weights pre-halved).

Cell tile CT (128, 640) f32 = [Ti Tf Tg | C | To]:
    tanh_ifg: CT[0:384]   = tanh(g[0:384])      (on the critical chain)
    tanh_o:   CT[512:640] = tanh(g[384:512])    (off-chain, overlaps DVE)
    uv = (CT[0:256]+1) * CT[256:512]            ([u|v] one 256-wide stt)
    C' = 0.5*v + u  -> next CT's C slot
    tc = tanh(0.5*C')
    H  = (To+1)*tc
Gate order on device: (i, f, g, o).

Prediction feedback (prev_y = mean_{t-1}) folded into rank-1 Wfb applied to
H2; means computed on host from exported H2.  Pred-phase matmul schedule:
only wfb@H2 and wi1@h1 sit on the serial chain; wi0@x + wh0@h1 are issued a
step early and b2m+wh1 run during cell1's elementwise chain.  Filler matmuls
keep the PE busy so its p-state ramps to 2.4 GHz instead of 1.2.
"""

import ml_dtypes
import numpy as np

BF16 = ml_dtypes.bfloat16

B = 1024
SEQ, PRED = 192, 96
W = SEQ + PRED  # 288
HID = 128
NCORES = 8
BS = B // NCORES  # 128
IN = 67
KX = IN + 2  # + ones row (bias1) + indicator row (pred feedback bias)
G4 = 4 * HID  # 512
# torch gate order (i, f, g, o) -> device order (i, f, o, g)
GATE_PERM = [0, 1, 3, 2]
HALVE = (0, 1, 2)  # i, f, o rows pre-halved (tanh trick); g untouched
X_CHUNK = 16  # scan steps per input-DMA chunk
WOFF = {"wi0": 0, "wh0": 512, "wi1": 1024, "wh1": 1536, "wfb": 2048,
        "b2m": 2560, "bones": 2688}
WCOLS = 2688 + 512  # 3200

# filler matmul column specs (one matmul per entry; tapered tails)
T_FILLA = [512, 384]          # teacher: after L2 openers
T_FILLB = [512, 384, 256, 128]  # teacher: after L1 groups
P_FILLA = [512, 512, 512, 384, 256, 128]  # pred: during cell1 chain
P_FILLB = [512, 512, 384, 256, 128]       # pred: during cell2 chain


def _perm_rows(w):
    """(4H, X) or (4H,) -> gate-permuted + i/f/o rows halved (tanh trick)."""
    w = w.reshape(4, HID, -1) if w.ndim == 2 else w.reshape(4, HID, 1)
    w = w[GATE_PERM].astype(np.float64).copy()
    for g in HALVE:
        w[g] *= 0.5
    return w  # (4, HID, X)


def _as_blocksT(w4):
    """(4, HID, K) -> (K, 4*HID) with gate blocks along columns (lhsT form)."""
    k = w4.shape[2]
    out = np.zeros((k, G4), np.float64)
    for g in range(4):
        out[:, g * HID:(g + 1) * HID] = w4[g].T
    return out


def host_prep(inputs):
    """All data-movement-only preprocessing + weight folding. Returns dict."""
    f32 = np.float32
    ge = np.asarray(inputs["given_enc"], f32)
    x_enc = np.asarray(inputs["x_enc"], f32)
    xm = np.asarray(inputs["x_mark_enc"], f32)
    mx = np.asarray(inputs["meta_x"], f32)
    tembs = [np.asarray(inputs[f"time_emb{i}"], f32) for i in range(3)]
    membs = [np.asarray(inputs[f"meta_emb{i}"], f32) for i in range(2)]

    tcat = ge[:, :, 4:7].astype(np.int32)
    time_feat = np.concatenate(
        [ge[:, :, :4]] + [tembs[i][tcat[:, :, i]] for i in range(3)], axis=-1
    )  # (B, W, 28)
    mcat = mx[:, 2:4].astype(np.int32)
    meta_feat = np.concatenate(
        [mx[:, :2]] + [membs[i][mcat[:, i]] for i in range(2)], axis=-1
    )  # (B, 34)

    nm = x_enc.mean(axis=1, keepdims=True)  # (B,1,1)
    xc = x_enc - nm
    ns = np.sqrt(xc.var(axis=1, keepdims=True) + 1e-5)
    xn = (xc / ns).astype(f32)  # (B, SEQ, 1)

    teacher = np.zeros((B, W, 1), f32)
    teacher[:, 0] = xn[:, 0]
    teacher[:, 1:SEQ] = xn[:, : SEQ - 1]
    ones = np.ones((B, W, 1), f32)
    ind = np.zeros((B, W, 1), f32)
    ind[:, SEQ:] = 1.0
    xfeat = np.concatenate(
        [teacher, time_feat, xm,
         np.broadcast_to(meta_feat[:, None, :], (B, W, 34)), ones, ind],
        axis=-1,
    )  # (B, W, 69)

    Wi0 = np.asarray(inputs["W_ih0"], np.float64)  # (512, 67)
    Wh0 = np.asarray(inputs["W_hh0"], np.float64)
    Wi1 = np.asarray(inputs["W_ih1"], np.float64)
    Wh1 = np.asarray(inputs["W_hh1"], np.float64)
    b1 = np.asarray(inputs["b_ih0"], np.float64) + np.asarray(inputs["b_hh0"], np.float64)
    b2 = np.asarray(inputs["b_ih1"], np.float64) + np.asarray(inputs["b_hh1"], np.float64)
    meanW = np.asarray(inputs["mean_W"], np.float64)  # (1, 128)
    mean_b = float(np.asarray(inputs["mean_b"]).reshape(()))

    wfb_full = Wi0[:, 0:1] @ (0.5 * meanW)  # consumes H2 = 2*h2
    bias_fb = Wi0[:, 0] * mean_b  # (512,)

    wi0T = _as_blocksT(_perm_rows(Wi0))  # (67, 512)
    wi0T_aug = np.zeros((KX, G4), np.float64)
    wi0T_aug[:IN] = wi0T
    wi0T_aug[IN] = _as_blocksT(_perm_rows(b1)).reshape(G4)  # ones row: bias1
    wi0T_aug[IN + 1] = _as_blocksT(_perm_rows(bias_fb)).reshape(G4)  # indicator
    wh0T = _as_blocksT(_perm_rows(Wh0) * 0.5)  # *0.5: h state is H = 2h
    wi1T = _as_blocksT(_perm_rows(Wi1) * 0.5)
    wh1T = _as_blocksT(_perm_rows(Wh1) * 0.5)
    wfbT = _as_blocksT(_perm_rows(wfb_full))  # (128, 512)

    b2m = _perm_rows(b2).reshape(4, HID)
    bones = np.zeros((4, G4), f32)
    for g in range(4):
        bones[g, g * HID:(g + 1) * HID] = 1.0

    # per-core transposed inputs: (KX, W*BS), feature on partitions
    xt_cores = []
    for c in range(NCORES):
        xf = xfeat[c * BS:(c + 1) * BS]  # (BS, W, KX)
        xt = np.ascontiguousarray(xf.transpose(2, 1, 0)).reshape(KX, W * BS)
        xt_cores.append(xt.astype(BF16))

    wconst = np.zeros((HID, WCOLS), BF16)
    wconst[:KX, WOFF["wi0"]:WOFF["wi0"] + G4] = wi0T_aug
    wconst[:, WOFF["wh0"]:WOFF["wh0"] + G4] = wh0T
    wconst[:, WOFF["wi1"]:WOFF["wi1"] + G4] = wi1T
    wconst[:, WOFF["wh1"]:WOFF["wh1"] + G4] = wh1T
    wconst[:, WOFF["wfb"]:WOFF["wfb"] + G4] = wfbT
    wconst[:4, WOFF["b2m"]:WOFF["b2m"] + HID] = b2m
    wconst[:4, WOFF["bones"]:WOFF["bones"] + G4] = bones

    return dict(
        xt_cores=xt_cores,
        wconst=wconst,
        weights=dict(
            wi0=wi0T_aug.astype(f32), wh0=wh0T.astype(f32),
            wi1=wi1T.astype(f32), wh1=wh1T.astype(f32),
            wfb=wfbT.astype(f32), b2m=b2m.astype(f32), bones=bones,
        ),
        meanW_h=(0.5 * meanW).astype(f32), mean_b=mean_b,
        norm_std=ns.astype(f32), norm_mean=nm.astype(f32),
    )


def host_post(h2_cores, prep):
    """h2_cores: list of (PRED, HID, BS) arrays of H2=2*h2. -> (B, PRED, 1)."""
    meanW_h = prep["meanW_h"][0]  # (HID,)
    out = np.empty((B, PRED, 1), np.float32)
    for c, h2 in enumerate(h2_cores):
        mn = np.einsum("h,thb->bt", meanW_h, h2.astype(np.float32)) + prep["mean_b"]
        out[c * BS:(c + 1) * BS, :, 0] = mn
    out = out * prep["norm_std"] + prep["norm_mean"]
    return out.astype(np.float32)


def build_bass():
    import concourse.bass as bass  # noqa: F401
    import concourse.tile as tile
    from concourse import bacc, mybir

    f32 = mybir.dt.float32
    bf16 = mybir.dt.bfloat16
    AF = mybir.ActivationFunctionType
    ALU = mybir.AluOpType
    OFF = 8  # teacher-phase layer-2 lag (decouples the two recurrence chains)

    nc = bacc.Bacc("TRN2", target_bir_lowering=False, num_devices=NCORES)
    xt_d = nc.dram_tensor("xt", [KX, W * BS], bf16, kind="ExternalInput")
    wc_d = nc.dram_tensor("wconst", [HID, WCOLS], bf16, kind="ExternalInput")
    h2out_d = nc.dram_tensor("h2out", [PRED, HID, BS], bf16, kind="ExternalOutput")

    with tile.TileContext(nc) as tc:
        with (
            tc.tile_pool(name="const", bufs=1) as const,
            tc.tile_pool(name="xin", bufs=3) as xin,
            tc.tile_pool(name="h1p", bufs=OFF + 3) as h1p,
            tc.tile_pool(name="st", bufs=3) as st,
            tc.tile_pool(name="ct1p", bufs=3) as ct1p,
            tc.tile_pool(name="ct2p", bufs=3) as ct2p,
            tc.tile_pool(name="work", bufs=3) as work,
            tc.tile_pool(name="ps", bufs=2, space="PSUM") as ps,
            tc.tile_pool(name="psf", bufs=1, space="PSUM") as psf,
        ):
            wc = const.tile([HID, WCOLS], bf16, tag="wc", name="wc")
            nc.sync.dma_start(out=wc, in_=wc_d[:, :])
            wt = {
                "wi0": wc[:KX, WOFF["wi0"]:WOFF["wi0"] + G4],
                "wh0": wc[:, WOFF["wh0"]:WOFF["wh0"] + G4],
                "wi1": wc[:, WOFF["wi1"]:WOFF["wi1"] + G4],
                "wh1": wc[:, WOFF["wh1"]:WOFF["wh1"] + G4],
                "wfb": wc[:, WOFF["wfb"]:WOFF["wfb"] + G4],
                "b2m": wc[:4, WOFF["b2m"]:WOFF["b2m"] + HID],
                "bones": wc[:4, WOFF["bones"]:WOFF["bones"] + G4],
            }

            def blk(ap, g):
                return ap[:, g * HID:(g + 1) * HID]

            h1 = h1p.tile([HID, BS], bf16, tag="h1", name="h1")
            nc.vector.memset(h1, 0.0)
            h2 = st.tile([HID, BS], bf16, tag="h2", name="h2")
            nc.vector.memset(h2, 0.0)
            h1_hist = {-1: h1}

            # cell tiles: [Ti Tf Tg | C | To], f32.  C slot of step t is
            # written by step t-1's c-op (or memset at t=0).
            ct1 = ct1p.tile([HID, 640], f32, tag="ct1", name="ct1")
            nc.vector.memset(ct1[:, 512:640], 0.0)
            ct2 = ct2p.tile([HID, 640], f32, tag="ct2", name="ct2")
            nc.vector.memset(ct2[:, 512:640], 0.0)

            # p-state ramp: >3us of continuous PE execution -> 2.4 GHz
            warm = psf.tile([HID, G4], f32, tag="fill", name="warm")
            for k in range(20):
                nc.tensor.matmul(warm, lhsT=wc[:, 0:HID], rhs=wc[:, 0:G4],
                                 start=(k == 0), stop=(k == 19))

            def fill(spec):
                for cols in spec:
                    ft = psf.tile([HID, G4], f32, tag="fill", name="ft")
                    nc.tensor.matmul(ft[:, :cols], lhsT=wc[:, 0:HID],
                                     rhs=wc[:, 0:cols], start=True, stop=True)

            def cell(g_ps, ct, ct_next, pool, tag, ve):
                """g_ps (128,512) PSUM gates [i f o g] -> h tile (bf16).
                CT layout: [Ti Tf To Tg | C], C slot = 512:640.
                ve: engine for the stt ops (nc.vector or nc.gpsimd)."""
                nc.scalar.activation(out=ct[:, 0:512], in_=g_ps[:, 0:512],
                                     func=AF.Tanh)
                uv = work.tile([HID, 256], f32, tag=f"uv{tag}", name=f"uv{tag}")
                ve.scalar_tensor_tensor(
                    out=uv, in0=ct[:, 0:256], scalar=1.0, in1=ct[:, 384:640],
                    op0=ALU.add, op1=ALU.mult)
                # C' = 0.5*v + u -> next step's C slot
                ve.scalar_tensor_tensor(
                    out=ct_next[:, 512:640], in0=uv[:, 128:256], scalar=0.5,
                    in1=uv[:, 0:128], op0=ALU.mult, op1=ALU.add)
                tc_ = work.tile([HID, BS], bf16, tag=f"tc{tag}", name=f"tc{tag}")
                nc.scalar.activation(out=tc_, in_=ct_next[:, 512:640],
                                     func=AF.Tanh, scale=0.5)
                h_new = pool.tile([HID, BS], bf16, tag=f"h{tag}", name=f"h{tag}")
                ve.scalar_tensor_tensor(
                    out=h_new, in0=ct[:, 256:384], scalar=1.0, in1=tc_,
                    op0=ALU.add, op1=ALU.mult)
                return h_new

            xt_sb = None

            def xcol_for(t):
                nonlocal xt_sb
                if t % X_CHUNK == 0:
                    nsteps = min(X_CHUNK, W - t)
                    xt_sb = xin.tile([KX, X_CHUNK * BS], bf16, tag="xt",
                                     name="xt_sb")
                    nc.sync.dma_start(out=xt_sb[:, :nsteps * BS],
                                      in_=xt_d[:, t * BS:(t + nsteps) * BS])
                return xt_sb[:, (t % X_CHUNK) * BS:(t % X_CHUNK + 1) * BS]

            # ---------------- teacher phase: L1 stream + L2 stream (lag OFF)
            # PE emit order per step: [wh1(j) close] [b2m+wi1(j+1) open]
            # [fillA] [wh0(i) close g1(i)] [wi0(i+1) open g1(i+1)] [fillB]
            g2_tiles = {}
            g1_tiles = {}
            # preamble: open g1(0)
            xcol = xcol_for(0)
            g1_tiles[0] = ps.tile([HID, G4], f32, tag="g1", name="g1")
            for g in range(4):
                nc.tensor.matmul(blk(g1_tiles[0], g), lhsT=blk(wt["wi0"], g),
                                 rhs=xcol, start=(g == 0), stop=False)
            for i in range(SEQ + OFF):
                j = i - OFF
                if j < 0:
                    fill([512] * 4)
                if j >= 0:
                    # late part: wh1@H2 closes g2(j) (waits h2(j-1))
                    g2 = g2_tiles.pop(j)
                    for g in range(4):
                        nc.tensor.matmul(blk(g2, g), lhsT=blk(wt["wh1"], g),
                                         rhs=h2, start=False, stop=(g == 3))
                jn = j + 1
                if 0 <= jn < SEQ:
                    # open g2(j+1): b2m + wi1@h1(j+1) (deps old; off-chain)
                    g2n = ps.tile([HID, G4], f32, tag="g2", name="g2")
                    g2_tiles[jn] = g2n
                    nc.tensor.matmul(g2n, lhsT=wt["b2m"], rhs=wt["bones"],
                                     start=True, stop=False)
                    for g in range(4):
                        nc.tensor.matmul(blk(g2n, g), lhsT=blk(wt["wi1"], g),
                                         rhs=h1_hist[jn], start=False,
                                         stop=False)
                if j >= 0:
                    ct2_next = ct2p.tile([HID, 640], f32, tag="ct2",
                                         name="ct2n")
                    h2 = cell(g2, ct2, ct2_next, st, "2", nc.vector)
                    ct2 = ct2_next
                fill(T_FILLA)
                if i < SEQ:
                    # close g1(i): wh0@h1(i-1)
                    g1 = g1_tiles.pop(i)
                    for g in range(4):
                        nc.tensor.matmul(blk(g1, g), lhsT=blk(wt["wh0"], g),
                                         rhs=h1_hist[i - 1], start=False,
                                         stop=(g == 3))
                    ct1_next = ct1p.tile([HID, 640], f32, tag="ct1",
                                         name="ct1n")
                    h1_hist[i] = cell(g1, ct1, ct1_next, h1p, "1", nc.vector)
                    ct1 = ct1_next
                    h1_hist.pop(i - OFF - 2, None)
                if i + 1 < SEQ:
                    # open g1(i+1): wi0@x (no recurrence dep)
                    xcol = xcol_for(i + 1)
                    g1n = ps.tile([HID, G4], f32, tag="g1", name="g1")
                    g1_tiles[i + 1] = g1n
                    for g in range(4):
                        nc.tensor.matmul(blk(g1n, g), lhsT=blk(wt["wi0"], g),
                                         rhs=xcol, start=(g == 0), stop=False)
                fill(T_FILLB)

            # ---------------- prediction phase
            h1 = h1_hist[SEQ - 1]
            # prefetch g1(SEQ) = wi0x + wh0@h1(SEQ-1)
            g1_next = ps.tile([HID, G4], f32, tag="g1", name="g1")
            xcol = xcol_for(SEQ)
            for g in range(4):
                nc.tensor.matmul(blk(g1_next, g), lhsT=blk(wt["wi0"], g),
                                 rhs=xcol, start=(g == 0), stop=False)
            for g in range(4):
                nc.tensor.matmul(blk(g1_next, g), lhsT=blk(wt["wh0"], g),
                                 rhs=h1, start=False, stop=False)

            for t in range(SEQ, W):
                # close g1(t): wfb@H2(t-1) — the only mm group on the chain
                g1 = g1_next
                for g in range(4):
                    nc.tensor.matmul(blk(g1, g), lhsT=blk(wt["wfb"], g),
                                     rhs=h2, start=False, stop=(g == 3))
                # g2(t) early part: deps ready now, runs during cell1 chain
                g2 = ps.tile([HID, G4], f32, tag="g2", name="g2")
                nc.tensor.matmul(g2, lhsT=wt["b2m"], rhs=wt["bones"],
                                 start=True, stop=False)
                for g in range(4):
                    nc.tensor.matmul(blk(g2, g), lhsT=blk(wt["wh1"], g),
                                     rhs=h2, start=False, stop=False)
                fill(P_FILLA)
                ct1_next = ct1p.tile([HID, 640], f32, tag="ct1", name="ct1n")
                h1 = cell(g1, ct1, ct1_next, h1p, "1", nc.vector)
                ct1 = ct1_next
                # close g2(t): wi1@h1(t)
                for g in range(4):
                    nc.tensor.matmul(blk(g2, g), lhsT=blk(wt["wi1"], g),
                                     rhs=h1, start=False, stop=(g == 3))
                # prefetch g1(t+1) + fillers: runs during cell2 chain
                if t + 1 < W:
                    g1_next = ps.tile([HID, G4], f32, tag="g1", name="g1")
                    xcol = xcol_for(t + 1)
                    for g in range(4):
                        nc.tensor.matmul(blk(g1_next, g), lhsT=blk(wt["wi0"], g),
                                         rhs=xcol, start=(g == 0), stop=False)
                    for g in range(4):
                        nc.tensor.matmul(blk(g1_next, g), lhsT=blk(wt["wh0"], g),
                                         rhs=h1, start=False, stop=False)
                fill(P_FILLB)
                ct2_next = ct2p.tile([HID, 640], f32, tag="ct2", name="ct2n")
                h2 = cell(g2, ct2, ct2_next, st, "2", nc.vector)
                ct2 = ct2_next
                nc.sync.dma_start(out=h2out_d[t - SEQ], in_=h2)
    nc.compile()
    return nc


_BASS_CACHE = {}


def _get_bass():
    if "nc" not in _BASS_CACHE:
        _BASS_CACHE["nc"] = build_bass()
    return _BASS_CACHE["nc"]


def run(inputs, trace=False):
    """Returns (output, BassKernelResults)."""
    from concourse.bass_utils import run_bass_kernel_spmd

    prep = host_prep(inputs)
    nc = _get_bass()
    in_maps = [{"xt": prep["xt_cores"][c], "wconst": prep["wconst"]}
               for c in range(NCORES)]
    res = run_bass_kernel_spmd(nc, in_maps, core_ids=list(range(NCORES)),
                               trace=trace)
    h2_cores = [r["h2out"] for r in res.results]
    return host_post(h2_cores, prep), res


def kernel(**inputs) -> np.ndarray:
    out, _ = run(inputs, trace=False)
    return out


# revision 12
# speedup vs baseline: 1.0104x; 1.0104x over previous
"""DeepAR (2-layer LSTM, B=1024, W=288, H=128) forward on 8 Trainium2 cores.

Pure data-parallel: batch 1024 -> 128 per core; weights replicated.

Device layout: (feature = partitions, batch = free).  sigmoid(x) =
(tanh(x/2)+1)/2 so ONE tanh table covers all gates; i/f/o weight rows are
pre-halved.  States: C = 2c, H = 2h (h-consuming weights pre-halved).

Cell tile CT (128, 640) f32 = [Ti Tf Tg | C | To]:
    tanh_ifg: CT[0:384]   = tanh(g[0:384])      (on the critical chain)
    tanh_o:   CT[512:640] = tanh(g[384:512])    (off-chain, overlaps DVE)
    uv = (CT[0:256]+1) * CT[256:512]            ([u|v] one 256-wide stt)
    C' = 0.5*v + u  -> next CT's C slot
    tc = tanh(0.5*C')
    H  = (To+1)*tc
Gate order on device: (i, f, g, o).

Prediction feedback (prev_y = mean_{t-1}) folded into rank-1 Wfb applied to
H2; means computed on host from exported H2.  Pred-phase matmul schedule:
only wfb@H2 and wi1@h1 sit on the serial chain; wi0@x + wh0@h1 are issued a
step early and b2m+wh1 run during cell1's elementwise chain.  Filler matmuls
keep the PE busy so its p-state ramps to 2.4 GHz instead of 1.2.
"""

import ml_dtypes
import numpy as np

BF16 = ml_dtypes.bfloat16

B = 1024
SEQ, PRED = 192, 96
W = SEQ + PRED  # 288
HID = 128
NCORES = 8
BS = B // NCORES  # 128
IN = 67
KX = IN + 2  # + ones row (bias1) + indicator row (pred feedback bias)
G4 = 4 * HID  # 512
# torch gate order (i, f, g, o) -> device order (i, f, g, o)
GATE_PERM = [0, 1, 2, 3]
HALVE = (0, 1, 3)  # i, f, o rows pre-halved (tanh trick); g untouched
X_CHUNK = 16  # scan steps per input-DMA chunk
WOFF = {"wi0": 0, "wh0": 512, "wi1": 1024, "wh1": 1536, "wfb": 2048,
        "b2m": 2560, "bones": 2688}
WCOLS = 2688 + 512  # 3200

# filler matmul column specs (one matmul per entry; tapered tails)
T_FILLA = [512, 384]          # teacher: after L2 openers
T_FILLB = [512, 384, 256, 128]  # teacher: after L1 groups
P_FILLA = [512, 512, 512, 384, 256, 128]  # pred: during cell1 chain
P_FILLB = [512, 512, 384, 256, 128]       # pred: during cell2 chain


def _perm_rows(w):
    """(4H, X) or (4H,) -> gate-permuted + i/f/o rows halved (tanh trick)."""
    w = w.reshape(4, HID, -1) if w.ndim == 2 else w.reshape(4, HID, 1)
    w = w[GATE_PERM].astype(np.float64).copy()
    for g in HALVE:
        w[g] *= 0.5
    return w  # (4, HID, X)


def _as_blocksT(w4):
    """(4, HID, K) -> (K, 4*HID) with gate blocks along columns (lhsT form)."""
    k = w4.shape[2]
    out = np.zeros((k, G4), np.float64)
    for g in range(4):
        out[:, g * HID:(g + 1) * HID] = w4[g].T
    return out


def host_prep(inputs):
    """All data-movement-only preprocessing + weight folding. Returns dict."""
    f32 = np.float32
    ge = np.asarray(inputs["given_enc"], f32)
    x_enc = np.asarray(inputs["x_enc"], f32)
    xm = np.asarray(inputs["x_mark_enc"], f32)
    mx = np.asarray(inputs["meta_x"], f32)
    tembs = [np.asarray(inputs[f"time_emb{i}"], f32) for i in range(3)]
    membs = [np.asarray(inputs[f"meta_emb{i}"], f32) for i in range(2)]

    tcat = ge[:, :, 4:7].astype(np.int32)
    time_feat = np.concatenate(
        [ge[:, :, :4]] + [tembs[i][tcat[:, :, i]] for i in range(3)], axis=-1
    )  # (B, W, 28)
    mcat = mx[:, 2:4].astype(np.int32)
    meta_feat = np.concatenate(
        [mx[:, :2]] + [membs[i][mcat[:, i]] for i in range(2)], axis=-1
    )  # (B, 34)

    nm = x_enc.mean(axis=1, keepdims=True)  # (B,1,1)
    xc = x_enc - nm
    ns = np.sqrt(xc.var(axis=1, keepdims=True) + 1e-5)
    xn = (xc / ns).astype(f32)  # (B, SEQ, 1)

    teacher = np.zeros((B, W, 1), f32)
    teacher[:, 0] = xn[:, 0]
    teacher[:, 1:SEQ] = xn[:, : SEQ - 1]
    ones = np.ones((B, W, 1), f32)
    ind = np.zeros((B, W, 1), f32)
    ind[:, SEQ:] = 1.0
    xfeat = np.concatenate(
        [teacher, time_feat, xm,
         np.broadcast_to(meta_feat[:, None, :], (B, W, 34)), ones, ind],
        axis=-1,
    )  # (B, W, 69)

    Wi0 = np.asarray(inputs["W_ih0"], np.float64)  # (512, 67)
    Wh0 = np.asarray(inputs["W_hh0"], np.float64)
    Wi1 = np.asarray(inputs["W_ih1"], np.float64)
    Wh1 = np.asarray(inputs["W_hh1"], np.float64)
    b1 = np.asarray(inputs["b_ih0"], np.float64) + np.asarray(inputs["b_hh0"], np.float64)
    b2 = np.asarray(inputs["b_ih1"], np.float64) + np.asarray(inputs["b_hh1"], np.float64)
    meanW = np.asarray(inputs["mean_W"], np.float64)  # (1, 128)
    mean_b = float(np.asarray(inputs["mean_b"]).reshape(()))

    wfb_full = Wi0[:, 0:1] @ (0.5 * meanW)  # consumes H2 = 2*h2
    bias_fb = Wi0[:, 0] * mean_b  # (512,)

    wi0T = _as_blocksT(_perm_rows(Wi0))  # (67, 512)
    wi0T_aug = np.zeros((KX, G4), np.float64)
    wi0T_aug[:IN] = wi0T
    wi0T_aug[IN] = _as_blocksT(_perm_rows(b1)).reshape(G4)  # ones row: bias1
    wi0T_aug[IN + 1] = _as_blocksT(_perm_rows(bias_fb)).reshape(G4)  # indicator
    wh0T = _as_blocksT(_perm_rows(Wh0) * 0.5)  # *0.5: h state is H = 2h
    wi1T = _as_blocksT(_perm_rows(Wi1) * 0.5)
    wh1T = _as_blocksT(_perm_rows(Wh1) * 0.5)
    wfbT = _as_blocksT(_perm_rows(wfb_full))  # (128, 512)

    b2m = _perm_rows(b2).reshape(4, HID)
    bones = np.zeros((4, G4), f32)
    for g in range(4):
        bones[g, g * HID:(g + 1) * HID] = 1.0

    # per-core transposed inputs: (KX, W*BS), feature on partitions
    xt_cores = []
    for c in range(NCORES):
        xf = xfeat[c * BS:(c + 1) * BS]  # (BS, W, KX)
        xt = np.ascontiguousarray(xf.transpose(2, 1, 0)).reshape(KX, W * BS)
        xt_cores.append(xt.astype(BF16))

    wconst = np.zeros((HID, WCOLS), BF16)
    wconst[:KX, WOFF["wi0"]:WOFF["wi0"] + G4] = wi0T_aug
    wconst[:, WOFF["wh0"]:WOFF["wh0"] + G4] = wh0T
    wconst[:, WOFF["wi1"]:WOFF["wi1"] + G4] = wi1T
    wconst[:, WOFF["wh1"]:WOFF["wh1"] + G4] = wh1T
    wconst[:, WOFF["wfb"]:WOFF["wfb"] + G4] = wfbT
    wconst[:4, WOFF["b2m"]:WOFF["b2m"] + HID] = b2m
    wconst[:4, WOFF["bones"]:WOFF["bones"] + G4] = bones

    return dict(
        xt_cores=xt_cores,
        wconst=wconst,
        weights=dict(
            wi0=wi0T_aug.astype(f32), wh0=wh0T.astype(f32),
            wi1=wi1T.astype(f32), wh1=wh1T.astype(f32),
            wfb=wfbT.astype(f32), b2m=b2m.astype(f32), bones=bones,
        ),
        meanW_h=(0.5 * meanW).astype(f32), mean_b=mean_b,
        norm_std=ns.astype(f32), norm_mean=nm.astype(f32),
    )


def host_post(h2_cores, prep):
    """h2_cores: list of (PRED, HID, BS) arrays of H2=2*h2. -> (B, PRED, 1)."""
    meanW_h = prep["meanW_h"][0]  # (HID,)
    out = np.empty((B, PRED, 1), np.float32)
    for c, h2 in enumerate(h2_cores):
        mn = np.einsum("h,thb->bt", meanW_h, h2.astype(np.float32)) + prep["mean_b"]
        out[c * BS:(c + 1) * BS, :, 0] = mn
    out = out * prep["norm_std"] + prep["norm_mean"]
    return out.astype(np.float32)


def build_bass():
    import concourse.bass as bass  # noqa: F401
    import concourse.tile as tile
    from concourse import bacc, mybir

    f32 = mybir.dt.float32
    bf16 = mybir.dt.bfloat16
    AF = mybir.ActivationFunctionType
    ALU = mybir.AluOpType
    OFF = 8  # teacher-phase layer-2 lag (decouples the two recurrence chains)

    nc = bacc.Bacc("TRN2", target_bir_lowering=False, num_devices=NCORES)
    xt_d = nc.dram_tensor("xt", [KX, W * BS], bf16, kind="ExternalInput")
    wc_d = nc.dram_tensor("wconst", [HID, WCOLS], bf16, kind="ExternalInput")
    h2out_d = nc.dram_tensor("h2out", [PRED, HID, BS], bf16, kind="ExternalOutput")

    with tile.TileContext(nc) as tc:
        with (
            tc.tile_pool(name="const", bufs=1) as const,
            tc.tile_pool(name="xin", bufs=3) as xin,
            tc.tile_pool(name="h1p", bufs=OFF + 3) as h1p,
            tc.tile_pool(name="st", bufs=3) as st,
            tc.tile_pool(name="ct1p", bufs=3) as ct1p,
            tc.tile_pool(name="ct2p", bufs=3) as ct2p,
            tc.tile_pool(name="work", bufs=3) as work,
            tc.tile_pool(name="ps", bufs=2, space="PSUM") as ps,
            tc.tile_pool(name="psf", bufs=1, space="PSUM") as psf,
        ):
            wc = const.tile([HID, WCOLS], bf16, tag="wc", name="wc")
            nc.sync.dma_start(out=wc, in_=wc_d[:, :])
            wt = {
                "wi0": wc[:KX, WOFF["wi0"]:WOFF["wi0"] + G4],
                "wh0": wc[:, WOFF["wh0"]:WOFF["wh0"] + G4],
                "wi1": wc[:, WOFF["wi1"]:WOFF["wi1"] + G4],
                "wh1": wc[:, WOFF["wh1"]:WOFF["wh1"] + G4],
                "wfb": wc[:, WOFF["wfb"]:WOFF["wfb"] + G4],
                "b2m": wc[:4, WOFF["b2m"]:WOFF["b2m"] + HID],
                "bones": wc[:4, WOFF["bones"]:WOFF["bones"] + G4],
            }

            def blk(ap, g):
                return ap[:, g * HID:(g + 1) * HID]

            h1 = h1p.tile([HID, BS], bf16, tag="h1", name="h1")
            nc.vector.memset(h1, 0.0)
            h2 = st.tile([HID, BS], bf16, tag="h2", name="h2")
            nc.vector.memset(h2, 0.0)
            h1_hist = {-1: h1}

            # cell tiles: [Ti Tf Tg | C | To], f32.  C slot of step t is
            # written by step t-1's c-op (or memset at t=0).
            ct1 = ct1p.tile([HID, 640], f32, tag="ct1", name="ct1")
            nc.vector.memset(ct1[:, 384:512], 0.0)
            ct2 = ct2p.tile([HID, 640], f32, tag="ct2", name="ct2")
            nc.vector.memset(ct2[:, 384:512], 0.0)

            # p-state ramp: >3us of continuous PE execution -> 2.4 GHz
            warm = psf.tile([HID, G4], f32, tag="fill", name="warm")
            for k in range(20):
                nc.tensor.matmul(warm, lhsT=wc[:, 0:HID], rhs=wc[:, 0:G4],
                                 start=(k == 0), stop=(k == 19))

            def fill(spec):
                for cols in spec:
                    ft = psf.tile([HID, G4], f32, tag="fill", name="ft")
                    nc.tensor.matmul(ft[:, :cols], lhsT=wc[:, 0:HID],
                                     rhs=wc[:, 0:cols], start=True, stop=True)

            def cell(g_ps, ct, ct_next, pool, tag, ve):
                """g_ps (128,512) PSUM gates [i f g o] -> h tile (bf16).
                CT layout: [Ti Tf Tg | C | To], C slot = 384:512.
                ve: engine for the stt ops."""
                nc.scalar.activation(out=ct[:, 0:384], in_=g_ps[:, 0:384],
                                     func=AF.Tanh)
                nc.scalar.activation(out=ct[:, 512:640], in_=g_ps[:, 384:512],
                                     func=AF.Tanh)
                uv = work.tile([HID, 256], f32, tag=f"uv{tag}", name=f"uv{tag}")
                ve.scalar_tensor_tensor(
                    out=uv, in0=ct[:, 0:256], scalar=1.0, in1=ct[:, 256:512],
                    op0=ALU.add, op1=ALU.mult)
                # C' = 0.5*v + u -> next step's C slot
                ve.scalar_tensor_tensor(
                    out=ct_next[:, 384:512], in0=uv[:, 128:256], scalar=0.5,
                    in1=uv[:, 0:128], op0=ALU.mult, op1=ALU.add)
                tc_ = work.tile([HID, BS], bf16, tag=f"tc{tag}", name=f"tc{tag}")
                nc.scalar.activation(out=tc_, in_=ct_next[:, 384:512],
                                     func=AF.Tanh, scale=0.5)
                h_new = pool.tile([HID, BS], bf16, tag=f"h{tag}", name=f"h{tag}")
                ve.scalar_tensor_tensor(
                    out=h_new, in0=ct[:, 512:640], scalar=1.0, in1=tc_,
                    op0=ALU.add, op1=ALU.mult)
                return h_new

            xt_sb = None

            def xcol_for(t):
                nonlocal xt_sb
                if t % X_CHUNK == 0:
                    nsteps = min(X_CHUNK, W - t)
                    xt_sb = xin.tile([KX, X_CHUNK * BS], bf16, tag="xt",
                                     name="xt_sb")
                    nc.sync.dma_start(out=xt_sb[:, :nsteps * BS],
                                      in_=xt_d[:, t * BS:(t + nsteps) * BS])
                return xt_sb[:, (t % X_CHUNK) * BS:(t % X_CHUNK + 1) * BS]

            # ---------------- teacher phase: L1 stream + L2 stream (lag OFF)
            # PE emit order per step: [wh1(j) close] [b2m+wi1(j+1) open]
            # [fillA] [wh0(i) close g1(i)] [wi0(i+1) open g1(i+1)] [fillB]
            g2_tiles = {}
            g1_tiles = {}
            # preamble: open g1(0)
            xcol = xcol_for(0)
            g1_tiles[0] = ps.tile([HID, G4], f32, tag="g1", name="g1")
            for g in range(4):
                nc.tensor.matmul(blk(g1_tiles[0], g), lhsT=blk(wt["wi0"], g),
                                 rhs=xcol, start=(g == 0), stop=False)
            for i in range(SEQ + OFF):
                j = i - OFF
                if j < 0:
                    fill([512] * 4)
                if j >= 0:
                    # late part: wh1@H2 closes g2(j) (waits h2(j-1))
                    g2 = g2_tiles.pop(j)
                    for g in range(4):
                        nc.tensor.matmul(blk(g2, g), lhsT=blk(wt["wh1"], g),
                                         rhs=h2, start=False, stop=(g == 3))
                jn = j + 1
                if 0 <= jn < SEQ:
                    # open g2(j+1): b2m + wi1@h1(j+1) (deps old; off-chain)
                    g2n = ps.tile([HID, G4], f32, tag="g2", name="g2")
                    g2_tiles[jn] = g2n
                    nc.tensor.matmul(g2n, lhsT=wt["b2m"], rhs=wt["bones"],
                                     start=True, stop=False)
                    for g in range(4):
                        nc.tensor.matmul(blk(g2n, g), lhsT=blk(wt["wi1"], g),
                                         rhs=h1_hist[jn], start=False,
                                         stop=False)
                if j >= 0:
                    ct2_next = ct2p.tile([HID, 640], f32, tag="ct2",
                                         name="ct2n")
                    h2 = cell(g2, ct2, ct2_next, st, "2", nc.vector)
                    ct2 = ct2_next
                fill(T_FILLA)
                if i < SEQ:
                    # close g1(i): wh0@h1(i-1)
                    g1 = g1_tiles.pop(i)
                    for g in range(4):
                        nc.tensor.matmul(blk(g1, g), lhsT=blk(wt["wh0"], g),
                                         rhs=h1_hist[i - 1], start=False,
                                         stop=(g == 3))
                    ct1_next = ct1p.tile([HID, 640], f32, tag="ct1",
                                         name="ct1n")
                    h1_hist[i] = cell(g1, ct1, ct1_next, h1p, "1", nc.vector)
                    ct1 = ct1_next
                    h1_hist.pop(i - OFF - 2, None)
                if i + 1 < SEQ:
                    # open g1(i+1): wi0@x (no recurrence dep)
                    xcol = xcol_for(i + 1)
                    g1n = ps.tile([HID, G4], f32, tag="g1", name="g1")
                    g1_tiles[i + 1] = g1n
                    for g in range(4):
                        nc.tensor.matmul(blk(g1n, g), lhsT=blk(wt["wi0"], g),
                                         rhs=xcol, start=(g == 0), stop=False)
                fill(T_FILLB)

            # ---------------- prediction phase
            h1 = h1_hist[SEQ - 1]
            # prefetch g1(SEQ) = wi0x + wh0@h1(SEQ-1)
            g1_next = ps.tile([HID, G4], f32, tag="g1", name="g1")
            xcol = xcol_for(SEQ)
            for g in range(4):
                nc.tensor.matmul(blk(g1_next, g), lhsT=blk(wt["wi0"], g),
                                 rhs=xcol, start=(g == 0), stop=False)
            for g in range(4):
                nc.tensor.matmul(blk(g1_next, g), lhsT=blk(wt["wh0"], g),
                                 rhs=h1, start=False, stop=False)

            for t in range(SEQ, W):
                # close g1(t): wfb@H2(t-1) — the only mm group on the chain
                g1 = g1_next
                for g in range(4):
                    nc.tensor.matmul(blk(g1, g), lhsT=blk(wt["wfb"], g),
                                     rhs=h2, start=False, stop=(g == 3))
                # g2(t) early part: deps ready now, runs during cell1 chain
                g2 = ps.tile([HID, G4], f32, tag="g2", name="g2")
                nc.tensor.matmul(g2, lhsT=wt["b2m"], rhs=wt["bones"],
                                 start=True, stop=False)
                for g in range(4):
                    nc.tensor.matmul(blk(g2, g), lhsT=blk(wt["wh1"], g),
                                     rhs=h2, start=False, stop=False)
                fill(P_FILLA)
                ct1_next = ct1p.tile([HID, 640], f32, tag="ct1", name="ct1n")
                h1 = cell(g1, ct1, ct1_next, h1p, "1", nc.vector)
                ct1 = ct1_next
                # close g2(t): wi1@h1(t)
                for g in range(4):
                    nc.tensor.matmul(blk(g2, g), lhsT=blk(wt["wi1"], g),
                                     rhs=h1, start=False, stop=(g == 3))
                # prefetch g1(t+1) + fillers: runs during cell2 chain
                if t + 1 < W:
                    g1_next = ps.tile([HID, G4], f32, tag="g1", name="g1")
                    xcol = xcol_for(t + 1)
                    for g in range(4):
                        nc.tensor.matmul(blk(g1_next, g), lhsT=blk(wt["wi0"], g),
                                         rhs=xcol, start=(g == 0), stop=False)
                    for g in range(4):
                        nc.tensor.matmul(blk(g1_next, g), lhsT=blk(wt["wh0"], g),
                                         rhs=h1, start=False, stop=False)
                fill(P_FILLB)
                ct2_next = ct2p.tile([HID, 640], f32, tag="ct2", name="ct2n")
                h2 = cell(g2, ct2, ct2_next, st, "2", nc.vector)
                ct2 = ct2_next
                nc.sync.dma_start(out=h2out_d[t - SEQ], in_=h2)
    nc.compile()
    return nc


_BASS_CACHE = {}


def _get_bass():
    if "nc" not in _BASS_CACHE:
        _BASS_CACHE["nc"] = build_bass()
    return _BASS_CACHE["nc"]


def run(inputs, trace=False):
    """Returns (output, BassKernelResults)."""
    from concourse.bass_utils import run_bass_kernel_spmd

    prep = host_prep(inputs)
    nc = _get_bass()
    in_maps = [{"xt": prep["xt_cores"][c], "wconst": prep["wconst"]}
               for c in range(NCORES)]
    res = run_bass_kernel_spmd(nc, in_maps, core_ids=list(range(NCORES)),
                               trace=trace)
    h2_cores = [r["h2out"] for r in res.results]
    return host_post(h2_cores, prep), res


def kernel(**inputs) -> np.ndarray:
    out, _ = run(inputs, trace=False)
    return out


# revision 13
# speedup vs baseline: 1.0536x; 1.0427x over previous
"""DeepAR (2-layer LSTM, B=1024, W=288, H=128) forward on 8 Trainium2 cores.

Pure data-parallel: batch 1024 -> 128 per core; weights replicated.

Device layout: (feature = partitions, batch = free).  sigmoid(x) =
(tanh(x/2)+1)/2 so ONE tanh table covers all gates; i/f/o weight rows are
pre-halved.  States: C = 2c, H = 2h (h-consuming weights pre-halved).

Cell tile CT (128, 640) f32 = [Ti Tf Tg | C | To]:
    tanh_ifg: CT[0:384]   = tanh(g[0:384])      (on the critical chain)
    tanh_o:   CT[512:640] = tanh(g[384:512])    (off-chain, overlaps DVE)
    uv = (CT[0:256]+1) * CT[256:512]            ([u|v] one 256-wide stt)
    C' = 0.5*v + u  -> next CT's C slot
    tc = tanh(0.5*C')
    H  = (To+1)*tc
Gate order on device: (i, f, g, o).

Prediction feedback (prev_y = mean_{t-1}) folded into rank-1 Wfb applied to
H2; means computed on host from exported H2.  Pred-phase matmul schedule:
only wfb@H2 and wi1@h1 sit on the serial chain; wi0@x + wh0@h1 are issued a
step early and b2m+wh1 run during cell1's elementwise chain.  Filler matmuls
keep the PE busy so its p-state ramps to 2.4 GHz instead of 1.2.
"""

import ml_dtypes
import numpy as np

BF16 = ml_dtypes.bfloat16

B = 1024
SEQ, PRED = 192, 96
W = SEQ + PRED  # 288
HID = 128
NCORES = 8
BS = B // NCORES  # 128
IN = 67
KX = IN + 2  # + ones row (bias1) + indicator row (pred feedback bias)
G4 = 4 * HID  # 512
# torch gate order (i, f, g, o) -> device order (i, f, g, o)
GATE_PERM = [0, 1, 2, 3]
HALVE = (0, 1, 3)  # i, f, o rows pre-halved (tanh trick); g untouched
X_CHUNK = 16  # scan steps per input-DMA chunk
WOFF = {"wi0": 0, "wh0": 512, "wi1": 1024, "wh1": 1536, "wfb": 2048,
        "b2m": 2560, "bones": 2688}
WCOLS = 2688 + 512  # 3200

# filler matmul column specs (one matmul per entry; tapered tails)
T_FILLA = [512] * 2           # teacher: after L2 openers
T_FILLB = [512] * 3             # teacher: after L1 groups
P_FILLA = [512] * 5             # pred: during cell1 chain
P_FILLB = [512] * 4             # pred: during cell2 chain


def _perm_rows(w):
    """(4H, X) or (4H,) -> gate-permuted + i/f/o rows halved (tanh trick)."""
    w = w.reshape(4, HID, -1) if w.ndim == 2 else w.reshape(4, HID, 1)
    w = w[GATE_PERM].astype(np.float64).copy()
    for g in HALVE:
        w[g] *= 0.5
    return w  # (4, HID, X)


def _as_blocksT(w4):
    """(4, HID, K) -> (K, 4*HID) with gate blocks along columns (lhsT form)."""
    k = w4.shape[2]
    out = np.zeros((k, G4), np.float64)
    for g in range(4):
        out[:, g * HID:(g + 1) * HID] = w4[g].T
    return out


def host_prep(inputs):
    """All data-movement-only preprocessing + weight folding. Returns dict."""
    f32 = np.float32
    ge = np.asarray(inputs["given_enc"], f32)
    x_enc = np.asarray(inputs["x_enc"], f32)
    xm = np.asarray(inputs["x_mark_enc"], f32)
    mx = np.asarray(inputs["meta_x"], f32)
    tembs = [np.asarray(inputs[f"time_emb{i}"], f32) for i in range(3)]
    membs = [np.asarray(inputs[f"meta_emb{i}"], f32) for i in range(2)]

    tcat = ge[:, :, 4:7].astype(np.int32)
    time_feat = np.concatenate(
        [ge[:, :, :4]] + [tembs[i][tcat[:, :, i]] for i in range(3)], axis=-1
    )  # (B, W, 28)
    mcat = mx[:, 2:4].astype(np.int32)
    meta_feat = np.concatenate(
        [mx[:, :2]] + [membs[i][mcat[:, i]] for i in range(2)], axis=-1
    )  # (B, 34)

    nm = x_enc.mean(axis=1, keepdims=True)  # (B,1,1)
    xc = x_enc - nm
    ns = np.sqrt(xc.var(axis=1, keepdims=True) + 1e-5)
    xn = (xc / ns).astype(f32)  # (B, SEQ, 1)

    teacher = np.zeros((B, W, 1), f32)
    teacher[:, 0] = xn[:, 0]
    teacher[:, 1:SEQ] = xn[:, : SEQ - 1]
    ones = np.ones((B, W, 1), f32)
    ind = np.zeros((B, W, 1), f32)
    ind[:, SEQ:] = 1.0
    xfeat = np.concatenate(
        [teacher, time_feat, xm,
         np.broadcast_to(meta_feat[:, None, :], (B, W, 34)), ones, ind],
        axis=-1,
    )  # (B, W, 69)

    Wi0 = np.asarray(inputs["W_ih0"], np.float64)  # (512, 67)
    Wh0 = np.asarray(inputs["W_hh0"], np.float64)
    Wi1 = np.asarray(inputs["W_ih1"], np.float64)
    Wh1 = np.asarray(inputs["W_hh1"], np.float64)
    b1 = np.asarray(inputs["b_ih0"], np.float64) + np.asarray(inputs["b_hh0"], np.float64)
    b2 = np.asarray(inputs["b_ih1"], np.float64) + np.asarray(inputs["b_hh1"], np.float64)
    meanW = np.asarray(inputs["mean_W"], np.float64)  # (1, 128)
    mean_b = float(np.asarray(inputs["mean_b"]).reshape(()))

    wfb_full = Wi0[:, 0:1] @ (0.5 * meanW)  # consumes H2 = 2*h2
    bias_fb = Wi0[:, 0] * mean_b  # (512,)

    wi0T = _as_blocksT(_perm_rows(Wi0))  # (67, 512)
    wi0T_aug = np.zeros((KX, G4), np.float64)
    wi0T_aug[:IN] = wi0T
    wi0T_aug[IN] = _as_blocksT(_perm_rows(b1)).reshape(G4)  # ones row: bias1
    wi0T_aug[IN + 1] = _as_blocksT(_perm_rows(bias_fb)).reshape(G4)  # indicator
    wh0T = _as_blocksT(_perm_rows(Wh0) * 0.5)  # *0.5: h state is H = 2h
    wi1T = _as_blocksT(_perm_rows(Wi1) * 0.5)
    wh1T = _as_blocksT(_perm_rows(Wh1) * 0.5)
    wfbT = _as_blocksT(_perm_rows(wfb_full))  # (128, 512)

    b2m = _perm_rows(b2).reshape(4, HID)
    bones = np.zeros((4, G4), f32)
    for g in range(4):
        bones[g, g * HID:(g + 1) * HID] = 1.0

    # per-core transposed inputs: (KX, W*BS), feature on partitions
    xt_cores = []
    for c in range(NCORES):
        xf = xfeat[c * BS:(c + 1) * BS]  # (BS, W, KX)
        xt = np.ascontiguousarray(xf.transpose(2, 1, 0)).reshape(KX, W * BS)
        xt_cores.append(xt.astype(BF16))

    wconst = np.zeros((HID, WCOLS), BF16)
    wconst[:KX, WOFF["wi0"]:WOFF["wi0"] + G4] = wi0T_aug
    wconst[:, WOFF["wh0"]:WOFF["wh0"] + G4] = wh0T
    wconst[:, WOFF["wi1"]:WOFF["wi1"] + G4] = wi1T
    wconst[:, WOFF["wh1"]:WOFF["wh1"] + G4] = wh1T
    wconst[:, WOFF["wfb"]:WOFF["wfb"] + G4] = wfbT
    wconst[:4, WOFF["b2m"]:WOFF["b2m"] + HID] = b2m
    wconst[:4, WOFF["bones"]:WOFF["bones"] + G4] = bones

    return dict(
        xt_cores=xt_cores,
        wconst=wconst,
        weights=dict(
            wi0=wi0T_aug.astype(f32), wh0=wh0T.astype(f32),
            wi1=wi1T.astype(f32), wh1=wh1T.astype(f32),
            wfb=wfbT.astype(f32), b2m=b2m.astype(f32), bones=bones,
        ),
        meanW_h=(0.5 * meanW).astype(f32), mean_b=mean_b,
        norm_std=ns.astype(f32), norm_mean=nm.astype(f32),
    )


def host_post(h2_cores, prep):
    """h2_cores: list of (PRED, HID, BS) arrays of H2=2*h2. -> (B, PRED, 1)."""
    meanW_h = prep["meanW_h"][0]  # (HID,)
    out = np.empty((B, PRED, 1), np.float32)
    for c, h2 in enumerate(h2_cores):
        mn = np.einsum("h,thb->bt", meanW_h, h2.astype(np.float32)) + prep["mean_b"]
        out[c * BS:(c + 1) * BS, :, 0] = mn
    out = out * prep["norm_std"] + prep["norm_mean"]
    return out.astype(np.float32)


def build_bass():
    import concourse.bass as bass  # noqa: F401
    import concourse.tile as tile
    from concourse import bacc, mybir

    f32 = mybir.dt.float32
    bf16 = mybir.dt.bfloat16
    AF = mybir.ActivationFunctionType
    ALU = mybir.AluOpType
    OFF = 8  # teacher-phase layer-2 lag (decouples the two recurrence chains)

    nc = bacc.Bacc("TRN2", target_bir_lowering=False, num_devices=NCORES)
    xt_d = nc.dram_tensor("xt", [KX, W * BS], bf16, kind="ExternalInput")
    wc_d = nc.dram_tensor("wconst", [HID, WCOLS], bf16, kind="ExternalInput")
    h2out_d = nc.dram_tensor("h2out", [PRED, HID, BS], bf16, kind="ExternalOutput")

    with tile.TileContext(nc) as tc:
        with (
            tc.tile_pool(name="const", bufs=1) as const,
            tc.tile_pool(name="xin", bufs=3) as xin,
            tc.tile_pool(name="h1p", bufs=OFF + 3) as h1p,
            tc.tile_pool(name="st", bufs=3) as st,
            tc.tile_pool(name="ct1p", bufs=3) as ct1p,
            tc.tile_pool(name="ct2p", bufs=3) as ct2p,
            tc.tile_pool(name="work", bufs=3) as work,
            tc.tile_pool(name="ps", bufs=2, space="PSUM") as ps,
            tc.tile_pool(name="psf", bufs=1, space="PSUM") as psf,
        ):
            wc = const.tile([HID, WCOLS], bf16, tag="wc", name="wc")
            nc.sync.dma_start(out=wc, in_=wc_d[:, :])
            wt = {
                "wi0": wc[:KX, WOFF["wi0"]:WOFF["wi0"] + G4],
                "wh0": wc[:, WOFF["wh0"]:WOFF["wh0"] + G4],
                "wi1": wc[:, WOFF["wi1"]:WOFF["wi1"] + G4],
                "wh1": wc[:, WOFF["wh1"]:WOFF["wh1"] + G4],
                "wfb": wc[:, WOFF["wfb"]:WOFF["wfb"] + G4],
                "b2m": wc[:4, WOFF["b2m"]:WOFF["b2m"] + HID],
                "bones": wc[:4, WOFF["bones"]:WOFF["bones"] + G4],
            }

            def blk(ap, g):
                return ap[:, g * HID:(g + 1) * HID]

            h1 = h1p.tile([HID, BS], bf16, tag="h1", name="h1")
            nc.vector.memset(h1, 0.0)
            h2 = st.tile([HID, BS], bf16, tag="h2", name="h2")
            nc.vector.memset(h2, 0.0)
            h1_hist = {-1: h1}

            # cell tiles: [Ti Tf Tg | C | To], f32.  C slot of step t is
            # written by step t-1's c-op (or memset at t=0).
            ct1 = ct1p.tile([HID, 640], f32, tag="ct1", name="ct1")
            nc.vector.memset(ct1[:, 384:512], 0.0)
            ct2 = ct2p.tile([HID, 640], f32, tag="ct2", name="ct2")
            nc.vector.memset(ct2[:, 384:512], 0.0)

            # p-state ramp: >3us of continuous PE execution -> 2.4 GHz
            warm = psf.tile([HID, G4], f32, tag="fill", name="warm")
            for k in range(20):
                nc.tensor.matmul(warm, lhsT=wc[:, 0:HID], rhs=wc[:, 0:G4],
                                 start=(k == 0), stop=(k == 19))

            def fill(spec):
                for cols in spec:
                    ft = psf.tile([HID, G4], f32, tag="fill", name="ft")
                    nc.tensor.matmul(ft[:, :cols], lhsT=wc[:, 0:HID],
                                     rhs=wc[:, 0:cols], start=True, stop=True)

            def cell(g_ps, ct, ct_next, pool, tag, ve):
                """g_ps (128,512) PSUM gates [i f g o] -> h tile (bf16).
                CT layout: [Ti Tf Tg | C | To], C slot = 384:512.
                ve: engine for the stt ops."""
                nc.scalar.activation(out=ct[:, 0:384], in_=g_ps[:, 0:384],
                                     func=AF.Tanh)
                nc.scalar.activation(out=ct[:, 512:640], in_=g_ps[:, 384:512],
                                     func=AF.Tanh)
                uv = work.tile([HID, 256], f32, tag=f"uv{tag}", name=f"uv{tag}")
                ve.scalar_tensor_tensor(
                    out=uv, in0=ct[:, 0:256], scalar=1.0, in1=ct[:, 256:512],
                    op0=ALU.add, op1=ALU.mult)
                # C' = 0.5*v + u -> next step's C slot
                ve.scalar_tensor_tensor(
                    out=ct_next[:, 384:512], in0=uv[:, 128:256], scalar=0.5,
                    in1=uv[:, 0:128], op0=ALU.mult, op1=ALU.add)
                tc_ = work.tile([HID, BS], bf16, tag=f"tc{tag}", name=f"tc{tag}")
                nc.scalar.activation(out=tc_, in_=ct_next[:, 384:512],
                                     func=AF.Tanh, scale=0.5)
                h_new = pool.tile([HID, BS], bf16, tag=f"h{tag}", name=f"h{tag}")
                ve.scalar_tensor_tensor(
                    out=h_new, in0=ct[:, 512:640], scalar=1.0, in1=tc_,
                    op0=ALU.add, op1=ALU.mult)
                return h_new

            xt_sb = None

            def xcol_for(t):
                nonlocal xt_sb
                if t % X_CHUNK == 0:
                    nsteps = min(X_CHUNK, W - t)
                    xt_sb = xin.tile([KX, X_CHUNK * BS], bf16, tag="xt",
                                     name="xt_sb")
                    nc.sync.dma_start(out=xt_sb[:, :nsteps * BS],
                                      in_=xt_d[:, t * BS:(t + nsteps) * BS])
                return xt_sb[:, (t % X_CHUNK) * BS:(t % X_CHUNK + 1) * BS]

            # ---------------- teacher phase: L1 stream + L2 stream (lag OFF)
            # PE emit order per step: [wh1(j) close] [b2m+wi1(j+1) open]
            # [fillA] [wh0(i) close g1(i)] [wi0(i+1) open g1(i+1)] [fillB]
            g2_tiles = {}
            g1_tiles = {}
            # preamble: open g1(0)
            xcol = xcol_for(0)
            g1_tiles[0] = ps.tile([HID, G4], f32, tag="g1", name="g1")
            for g in range(4):
                nc.tensor.matmul(blk(g1_tiles[0], g), lhsT=blk(wt["wi0"], g),
                                 rhs=xcol, start=(g == 0), stop=False)
            for i in range(SEQ + OFF):
                j = i - OFF
                if j < 0:
                    fill([512] * 4)
                if j >= 0:
                    # late part: wh1@H2 closes g2(j) (waits h2(j-1))
                    g2 = g2_tiles.pop(j)
                    for g in range(4):
                        nc.tensor.matmul(blk(g2, g), lhsT=blk(wt["wh1"], g),
                                         rhs=h2, start=False, stop=(g == 3))
                jn = j + 1
                if 0 <= jn < SEQ:
                    # open g2(j+1): b2m + wi1@h1(j+1) (deps old; off-chain)
                    g2n = ps.tile([HID, G4], f32, tag="g2", name="g2")
                    g2_tiles[jn] = g2n
                    nc.tensor.matmul(g2n, lhsT=wt["b2m"], rhs=wt["bones"],
                                     start=True, stop=False)
                    for g in range(4):
                        nc.tensor.matmul(blk(g2n, g), lhsT=blk(wt["wi1"], g),
                                         rhs=h1_hist[jn], start=False,
                                         stop=False)
                if j >= 0:
                    ct2_next = ct2p.tile([HID, 640], f32, tag="ct2",
                                         name="ct2n")
                    h2 = cell(g2, ct2, ct2_next, st, "2", nc.vector)
                    ct2 = ct2_next
                fill(T_FILLA)
                if i < SEQ:
                    # close g1(i): wh0@h1(i-1)
                    g1 = g1_tiles.pop(i)
                    for g in range(4):
                        nc.tensor.matmul(blk(g1, g), lhsT=blk(wt["wh0"], g),
                                         rhs=h1_hist[i - 1], start=False,
                                         stop=(g == 3))
                    ct1_next = ct1p.tile([HID, 640], f32, tag="ct1",
                                         name="ct1n")
                    h1_hist[i] = cell(g1, ct1, ct1_next, h1p, "1", nc.vector)
                    ct1 = ct1_next
                    h1_hist.pop(i - OFF - 2, None)
                if i + 1 < SEQ:
                    # open g1(i+1): wi0@x (no recurrence dep)
                    xcol = xcol_for(i + 1)
                    g1n = ps.tile([HID, G4], f32, tag="g1", name="g1")
                    g1_tiles[i + 1] = g1n
                    for g in range(4):
                        nc.tensor.matmul(blk(g1n, g), lhsT=blk(wt["wi0"], g),
                                         rhs=xcol, start=(g == 0), stop=False)
                fill(T_FILLB)

            # ---------------- prediction phase
            h1 = h1_hist[SEQ - 1]
            # prefetch g1(SEQ) = wi0x + wh0@h1(SEQ-1)
            g1_next = ps.tile([HID, G4], f32, tag="g1", name="g1")
            xcol = xcol_for(SEQ)
            for g in range(4):
                nc.tensor.matmul(blk(g1_next, g), lhsT=blk(wt["wi0"], g),
                                 rhs=xcol, start=(g == 0), stop=False)
            for g in range(4):
                nc.tensor.matmul(blk(g1_next, g), lhsT=blk(wt["wh0"], g),
                                 rhs=h1, start=False, stop=False)

            for t in range(SEQ, W):
                # close g1(t): wfb@H2(t-1) — the only mm group on the chain
                g1 = g1_next
                for g in range(4):
                    nc.tensor.matmul(blk(g1, g), lhsT=blk(wt["wfb"], g),
                                     rhs=h2, start=False, stop=(g == 3))
                # g2(t) early part: deps ready now, runs during cell1 chain
                g2 = ps.tile([HID, G4], f32, tag="g2", name="g2")
                nc.tensor.matmul(g2, lhsT=wt["b2m"], rhs=wt["bones"],
                                 start=True, stop=False)
                for g in range(4):
                    nc.tensor.matmul(blk(g2, g), lhsT=blk(wt["wh1"], g),
                                     rhs=h2, start=False, stop=False)
                fill(P_FILLA)
                ct1_next = ct1p.tile([HID, 640], f32, tag="ct1", name="ct1n")
                h1 = cell(g1, ct1, ct1_next, h1p, "1", nc.vector)
                ct1 = ct1_next
                # close g2(t): wi1@h1(t)
                for g in range(4):
                    nc.tensor.matmul(blk(g2, g), lhsT=blk(wt["wi1"], g),
                                     rhs=h1, start=False, stop=(g == 3))
                # prefetch g1(t+1) + fillers: runs during cell2 chain
                if t + 1 < W:
                    g1_next = ps.tile([HID, G4], f32, tag="g1", name="g1")
                    xcol = xcol_for(t + 1)
                    for g in range(4):
                        nc.tensor.matmul(blk(g1_next, g), lhsT=blk(wt["wi0"], g),
                                         rhs=xcol, start=(g == 0), stop=False)
                    for g in range(4):
                        nc.tensor.matmul(blk(g1_next, g), lhsT=blk(wt["wh0"], g),
                                         rhs=h1, start=False, stop=False)
                fill(P_FILLB)
                ct2_next = ct2p.tile([HID, 640], f32, tag="ct2", name="ct2n")
                h2 = cell(g2, ct2, ct2_next, st, "2", nc.vector)
                ct2 = ct2_next
                nc.sync.dma_start(out=h2out_d[t - SEQ], in_=h2)
    nc.compile()
    return nc


_BASS_CACHE = {}


def _get_bass():
    if "nc" not in _BASS_CACHE:
        _BASS_CACHE["nc"] = build_bass()
    return _BASS_CACHE["nc"]


def run(inputs, trace=False):
    """Returns (output, BassKernelResults)."""
    from concourse.bass_utils import run_bass_kernel_spmd

    prep = host_prep(inputs)
    nc = _get_bass()
    in_maps = [{"xt": prep["xt_cores"][c], "wconst": prep["wconst"]}
               for c in range(NCORES)]
    res = run_bass_kernel_spmd(nc, in_maps, core_ids=list(range(NCORES)),
                               trace=trace)
    h2_cores = [r["h2out"] for r in res.results]
    return host_post(h2_cores, prep), res


def kernel(**inputs) -> np.ndarray:
    out, _ = run(inputs, trace=False)
    return out
